# revision 1
# baseline (speedup 1.0000x reference)
"""MoE router kernel for Trainium2 (8 NeuronCores, data-parallel over tokens).

Computes, for x:[32768,2048] f32 and gate_w:[2048,64] f32:
  logits = x @ gate_w
  top2 values/indices, top2 softmax weights
  probs = softmax(logits); load_balance_loss = E * sum(f * P)

Sharding: token dim split 8 ways (4096 tokens/core); gate replicated.
Per-core partials (tokens_per_expert, sum of probs) are reduced on host.

Self-contained: hardcodes shapes; imports only concourse (on PYTHONPATH).
"""

import numpy as np

N_TOKENS = 32768
HIDDEN = 2048
NUM_EXPERTS = 64
TOP_K = 2

N_CORES = 8
NP = N_TOKENS // N_CORES  # tokens per core = 4096
KCH = HIDDEN // 128       # hidden chunks of 128 = 16
BLK = 512                 # tokens per DMA block
NBLK = NP // BLK          # 8
TPB = BLK // 128          # matmul tiles per block = 4

_CACHE = {}


def _build_nc():
    import concourse.bacc as bacc
    import concourse.tile as tile
    from concourse import mybir

    f32 = mybir.dt.float32
    u32 = mybir.dt.uint32
    Alu = mybir.AluOpType
    Act = mybir.ActivationFunctionType

    nc = bacc.Bacc("TRN2", target_bir_lowering=False, debug=False)

    xT = nc.dram_tensor("xT", [HIDDEN, NP], f32, kind="ExternalInput").ap()
    gw = nc.dram_tensor("gw", [HIDDEN, NUM_EXPERTS], f32, kind="ExternalInput").ap()
    w_out = nc.dram_tensor("w_out", [NP, TOP_K], f32, kind="ExternalOutput").ap()
    i_out = nc.dram_tensor("i_out", [NP, TOP_K], u32, kind="ExternalOutput").ap()
    p_out = nc.dram_tensor("p_out", [1, 2 * NUM_EXPERTS], f32, kind="ExternalOutput").ap()

    with tile.TileContext(nc) as tc:
        with tc.tile_pool(name="xpool", bufs=3) as xpool, \
             tc.tile_pool(name="cpool", bufs=1) as cpool, \
             tc.tile_pool(name="lpool", bufs=4) as lpool, \
             tc.tile_pool(name="spool", bufs=6) as spool, \
             tc.tile_pool(name="stpool", bufs=3) as stpool, \
             tc.tile_pool(name="pspool", bufs=6, space="PSUM") as pspool, \
             tc.tile_pool(name="pfpool", bufs=1, space="PSUM") as pfpool:

            # ---- constants ----
            w_sb = cpool.tile([128, KCH, NUM_EXPERTS], f32)
            nc.sync.dma_start(out=w_sb, in_=gw.rearrange("(K p) e -> p K e", p=128))

            iota = cpool.tile([128, NUM_EXPERTS], u32)
            nc.gpsimd.iota(iota, pattern=[[1, NUM_EXPERTS]], base=0,
                           channel_multiplier=0)

            # acc[:, :64] = routed-count one-hots, acc[:, 64:] = prob sums
            acc = cpool.tile([128, 2 * NUM_EXPERTS], f32)
            nc.vector.memset(acc, 0.0)
            accC = acc[:, 0:NUM_EXPERTS]
            accP = acc[:, NUM_EXPERTS:2 * NUM_EXPERTS]

            ones = cpool.tile([128, 1], f32)
            nc.vector.memset(ones, 1.0)

            xTv = xT.rearrange("(K p) t -> p K t", p=128)

            for b in range(NBLK):
                t0 = b * BLK
                xblk = xpool.tile([128, KCH, BLK], f32, tag="xblk")
                nc.sync.dma_start(out=xblk, in_=xTv[:, :, t0:t0 + BLK])

                wstage = stpool.tile([128, TPB, TOP_K], f32, tag="wstage")
                istage = stpool.tile([128, TPB, TOP_K], u32, tag="istage")

                for j in range(TPB):
                    # logits[tok, e] for 128 tokens, accumulated over 16 hidden chunks
                    ps = pspool.tile([128, NUM_EXPERTS], f32, tag="ps")
                    for k in range(KCH):
                        nc.tensor.matmul(
                            ps,
                            lhsT=xblk[:, k, j * 128:(j + 1) * 128],
                            rhs=w_sb[:, k, :],
                            start=(k == 0),
                            stop=(k == KCH - 1),
                        )

                    logit = lpool.tile([128, NUM_EXPERTS], f32, tag="logit")
                    nc.scalar.activation(logit, ps, Act.Copy)

                    mx = spool.tile([128, 8], f32, tag="mx")
                    nc.vector.max(out=mx, in_=logit)
                    mi = spool.tile([128, 8], u32, tag="mi")
                    nc.vector.max_index(out=mi, in_max=mx, in_values=logit)

                    negm = spool.tile([128, 1], f32, tag="negm")
                    nc.vector.tensor_scalar_mul(negm, mx[:, 0:1], -1.0)

                    # full-row softmax numerator + row sum (for probs partials)
                    ex = lpool.tile([128, NUM_EXPERTS], f32, tag="ex")
                    s = spool.tile([128, 1], f32, tag="s")
                    nc.scalar.activation(ex, logit, Act.Exp, bias=negm, scale=1.0,
                                         accum_out=s)

                    # top-2 softmax: [1, exp(m2-m1)] / (1 + exp(m2-m1))
                    pairex = spool.tile([128, TOP_K], f32, tag="pairex")
                    denom = spool.tile([128, 1], f32, tag="denom")
                    nc.scalar.activation(pairex, mx[:, 0:TOP_K], Act.Exp, bias=negm,
                                         scale=1.0, accum_out=denom)
                    rden = spool.tile([128, 1], f32, tag="rden")
                    nc.vector.reciprocal(rden, denom)
                    nc.vector.tensor_scalar_mul(wstage[:, j, :], pairex, rden)

                    rs = spool.tile([128, 1], f32, tag="rs")
                    nc.vector.reciprocal(rs, s)
                    # accP += ex * (1/s)
                    nc.vector.scalar_tensor_tensor(
                        out=accP, in0=ex, scalar=rs, in1=accP,
                        op0=Alu.mult, op1=Alu.add)
                    # accC += onehot(i1) ; accC += onehot(i2)
                    nc.vector.scalar_tensor_tensor(
                        out=accC, in0=iota, scalar=mi[:, 0:1], in1=accC,
                        op0=Alu.is_equal, op1=Alu.add)
                    nc.vector.scalar_tensor_tensor(
                        out=accC, in0=iota, scalar=mi[:, 1:2], in1=accC,
                        op0=Alu.is_equal, op1=Alu.add)

                    nc.vector.tensor_copy(istage[:, j, :], mi[:, 0:TOP_K])

                nc.sync.dma_start(
                    out=w_out[t0:t0 + BLK, :].rearrange("(j p) k -> p j k", p=128),
                    in_=wstage)
                nc.sync.dma_start(
                    out=i_out[t0:t0 + BLK, :].rearrange("(j p) k -> p j k", p=128),
                    in_=istage)

            # column-sum acc over the 128 partitions: [1, 128] = ones.T @ acc
            pps = pfpool.tile([1, 2 * NUM_EXPERTS], f32, tag="pps")
            nc.tensor.matmul(pps, lhsT=ones, rhs=acc, start=True, stop=True)
            psb = cpool.tile([1, 2 * NUM_EXPERTS], f32)
            nc.scalar.activation(psb, pps, Act.Copy)
            nc.sync.dma_start(out=p_out, in_=psb)

    nc.compile()
    return nc


def _get_nc():
    if "nc" not in _CACHE:
        _CACHE["nc"] = _build_nc()
    return _CACHE["nc"]


def _run(x, gate_w, trace=False, trace_cores=None):
    from concourse import bass_utils

    nc = _get_nc()
    x = np.ascontiguousarray(np.asarray(x, dtype=np.float32))
    gate_w = np.ascontiguousarray(np.asarray(gate_w, dtype=np.float32))

    xT = np.ascontiguousarray(x.T)  # [HIDDEN, N_TOKENS]
    in_maps = [
        {"xT": np.ascontiguousarray(xT[:, c * NP:(c + 1) * NP]), "gw": gate_w}
        for c in range(N_CORES)
    ]
    kw = {}
    if trace:
        kw = {"trace": True,
              "trace_cores": trace_cores if trace_cores is not None else [0]}
    res = bass_utils.run_bass_kernel_spmd(nc, in_maps, list(range(N_CORES)), **kw)
    outs = res.results

    w = np.concatenate([outs[c]["w_out"] for c in range(N_CORES)], axis=0)
    idx = np.concatenate([outs[c]["i_out"] for c in range(N_CORES)],
                         axis=0).astype(np.int32)
    partials = np.stack([outs[c]["p_out"][0] for c in range(N_CORES)], axis=0)
    totals = partials.sum(axis=0, dtype=np.float64)
    counts = totals[:NUM_EXPERTS]
    psum = totals[NUM_EXPERTS:]
    f = counts / float(N_TOKENS)
    P = psum / float(N_TOKENS)
    loss = np.float32(NUM_EXPERTS * np.sum(f * P))
    return (w.astype(np.float32), idx, loss), res


def kernel(x, gate_w):
    (w, idx, loss), _ = _run(x, gate_w)
    return w, idx, loss


# revision 3
# speedup vs baseline: 1.8968x; 1.8968x over previous
"""MoE router kernel for Trainium2 (8 NeuronCores, data-parallel over tokens).

Computes, for x:[32768,2048] f32 and gate_w:[2048,64] f32:
  logits = x @ gate_w
  top2 values/indices, top2 softmax weights
  probs = softmax(logits); load_balance_loss = E * sum(f * P)

Sharding: token dim split 8 ways (4096 tokens/core); gate replicated.
Per-core partials (tokens_per_expert, sum of probs) are reduced on host.

The fp32 matmul is emulated at ~fp32 precision with a bf16 hi/lo split:
  x = xh + xl, w = wh + wl, logits = xh@wh + xh@wl + xl@wh
computed as xh@[wh|wl] (N=128) + xl@wh (N=64) accumulated in one PSUM
tile; the two column halves are summed on the vector engine.

Tokens are permuted host-side so that mm-tile j / partition p holds local
token p*32+j — this makes the [4096,2] outputs a single contiguous-ish
DMA (256B runs per partition) instead of 8-byte scatter.

Self-contained: hardcodes shapes; imports only concourse (on PYTHONPATH).
"""

import numpy as np

N_TOKENS = 32768
HIDDEN = 2048
NUM_EXPERTS = 64
TOP_K = 2

N_CORES = 8
NP = N_TOKENS // N_CORES  # tokens per core = 4096
KCH = HIDDEN // 128       # hidden chunks of 128 = 16
BLK = 512                 # tokens per DMA block
NBLK = NP // BLK          # 8
TPB = BLK // 128          # matmul tiles per block = 4
NTILE = NP // 128         # 32 mm-tiles per core

_CACHE = {}


def _build_nc():
    import concourse.bacc as bacc
    import concourse.tile as tile
    from concourse import mybir

    f32 = mybir.dt.float32
    bf16 = mybir.dt.bfloat16
    u32 = mybir.dt.uint32
    Alu = mybir.AluOpType
    Act = mybir.ActivationFunctionType

    nc = bacc.Bacc("TRN2", target_bir_lowering=False, debug=False)

    xhl = nc.dram_tensor("xhl", [2, HIDDEN, NP], bf16, kind="ExternalInput").ap()
    whl = nc.dram_tensor("whl", [HIDDEN, 2 * NUM_EXPERTS], bf16,
                         kind="ExternalInput").ap()
    w_out = nc.dram_tensor("w_out", [NP, TOP_K], f32, kind="ExternalOutput").ap()
    i_out = nc.dram_tensor("i_out", [NP, TOP_K], u32, kind="ExternalOutput").ap()
    p_out = nc.dram_tensor("p_out", [1, 2 * NUM_EXPERTS], f32,
                           kind="ExternalOutput").ap()

    with tile.TileContext(nc) as tc:
        with tc.tile_pool(name="xpool", bufs=3) as xpool, \
             tc.tile_pool(name="cpool", bufs=1) as cpool, \
             tc.tile_pool(name="lpool", bufs=4) as lpool, \
             tc.tile_pool(name="spool", bufs=6) as spool, \
             tc.tile_pool(name="pspool", bufs=6, space="PSUM") as pspool, \
             tc.tile_pool(name="pfpool", bufs=1, space="PSUM") as pfpool:

            # ---- constants ----
            w_sb = cpool.tile([128, KCH, 2 * NUM_EXPERTS], bf16)
            nc.sync.dma_start(out=w_sb, in_=whl.rearrange("(K p) e -> p K e", p=128))

            iota = cpool.tile([128, NUM_EXPERTS], u32)
            nc.gpsimd.iota(iota, pattern=[[1, NUM_EXPERTS]], base=0,
                           channel_multiplier=0)

            # acc[:, :64] = routed-count one-hots, acc[:, 64:] = prob sums
            acc = cpool.tile([128, 2 * NUM_EXPERTS], f32)
            nc.vector.memset(acc, 0.0)
            accC = acc[:, 0:NUM_EXPERTS]
            accP = acc[:, NUM_EXPERTS:2 * NUM_EXPERTS]

            ones = cpool.tile([128, 1], f32)
            nc.vector.memset(ones, 1.0)

            # all-core output staging (written tile by tile, stored once)
            wstage = cpool.tile([128, NTILE, TOP_K], f32)
            istage = cpool.tile([128, NTILE, TOP_K], u32)

            xv = xhl.rearrange("two (K p) t -> p two K t", p=128)

            for b in range(NBLK):
                t0 = b * BLK
                xblk = xpool.tile([128, 2, KCH, BLK], bf16, tag="xblk")
                nc.sync.dma_start(out=xblk, in_=xv[:, :, :, t0:t0 + BLK])

                for jj in range(TPB):
                    j = b * TPB + jj
                    ts = jj * 128
                    # psum cols 0:64 <- xh@wh + xl@wh ; cols 64:128 <- xh@wl
                    ps = pspool.tile([128, 2 * NUM_EXPERTS], f32, tag="ps")
                    for k in range(KCH):
                        nc.tensor.matmul(
                            ps,
                            lhsT=xblk[:, 0, k, ts:ts + 128],
                            rhs=w_sb[:, k, :],
                            start=(k == 0),
                            stop=False,
                        )
                    for k in range(KCH):
                        nc.tensor.matmul(
                            ps[:, 0:NUM_EXPERTS],
                            lhsT=xblk[:, 1, k, ts:ts + 128],
                            rhs=w_sb[:, k, 0:NUM_EXPERTS],
                            start=False,
                            stop=(k == KCH - 1),
                        )

                    hi = lpool.tile([128, NUM_EXPERTS], f32, tag="hi")
                    nc.scalar.activation(hi, ps[:, NUM_EXPERTS:2 * NUM_EXPERTS],
                                         Act.Copy)
                    logit = lpool.tile([128, NUM_EXPERTS], f32, tag="logit")
                    nc.vector.tensor_add(logit, ps[:, 0:NUM_EXPERTS], hi)

                    mx = spool.tile([128, 8], f32, tag="mx")
                    nc.vector.max(out=mx, in_=logit)
                    mi = spool.tile([128, 8], u32, tag="mi")
                    nc.vector.max_index(out=mi, in_max=mx, in_values=logit)

                    negm = spool.tile([128, 1], f32, tag="negm")
                    nc.vector.tensor_scalar_mul(negm, mx[:, 0:1], -1.0)

                    # full-row softmax numerator + row sum (for probs partials)
                    ex = lpool.tile([128, NUM_EXPERTS], f32, tag="ex")
                    s = spool.tile([128, 1], f32, tag="s")
                    nc.scalar.activation(ex, logit, Act.Exp, bias=negm, scale=1.0,
                                         accum_out=s)

                    # top-2 softmax: [1, exp(m2-m1)] / (1 + exp(m2-m1))
                    pairex = spool.tile([128, TOP_K], f32, tag="pairex")
                    denom = spool.tile([128, 1], f32, tag="denom")
                    nc.scalar.activation(pairex, mx[:, 0:TOP_K], Act.Exp, bias=negm,
                                         scale=1.0, accum_out=denom)
                    rden = spool.tile([128, 1], f32, tag="rden")
                    nc.vector.reciprocal(rden, denom)
                    nc.vector.tensor_scalar_mul(wstage[:, j, :], pairex, rden)

                    rs = spool.tile([128, 1], f32, tag="rs")
                    nc.vector.reciprocal(rs, s)
                    # accP += ex * (1/s)
                    nc.vector.scalar_tensor_tensor(
                        out=accP, in0=ex, scalar=rs, in1=accP,
                        op0=Alu.mult, op1=Alu.add)
                    # accC += onehot(i1) ; accC += onehot(i2)
                    nc.vector.scalar_tensor_tensor(
                        out=accC, in0=iota, scalar=mi[:, 0:1], in1=accC,
                        op0=Alu.is_equal, op1=Alu.add)
                    nc.vector.scalar_tensor_tensor(
                        out=accC, in0=iota, scalar=mi[:, 1:2], in1=accC,
                        op0=Alu.is_equal, op1=Alu.add)

                    nc.vector.tensor_copy(istage[:, j, :], mi[:, 0:TOP_K])

            # single staged stores; local token id = p*32 + j
            nc.sync.dma_start(
                out=w_out.rearrange("(p j) k -> p j k", p=128), in_=wstage)
            nc.sync.dma_start(
                out=i_out.rearrange("(p j) k -> p j k", p=128), in_=istage)

            # column-sum acc over the 128 partitions: [1, 128] = ones.T @ acc
            pps = pfpool.tile([1, 2 * NUM_EXPERTS], f32, tag="pps")
            nc.tensor.matmul(pps, lhsT=ones, rhs=acc, start=True, stop=True)
            psb = cpool.tile([1, 2 * NUM_EXPERTS], f32)
            nc.scalar.activation(psb, pps, Act.Copy)
            nc.sync.dma_start(out=p_out, in_=psb)

    nc.compile()
    return nc


def _get_nc():
    if "nc" not in _CACHE:
        _CACHE["nc"] = _build_nc()
    return _CACHE["nc"]


def _prep_core(x_shard):
    """x_shard [NP, HIDDEN] f32 -> xhl [2, HIDDEN, NP] bf16, token-permuted.

    Permutation: kernel column index j*128+p holds local token p*32+j.
    """
    import ml_dtypes

    bf = ml_dtypes.bfloat16
    xt = np.ascontiguousarray(x_shard.T)                # [H, NP] cols = tokens
    xt = xt.reshape(HIDDEN, 128, NTILE).transpose(0, 2, 1).reshape(HIDDEN, NP)
    xh = xt.astype(bf)
    xl = (xt - xh.astype(np.float32)).astype(bf)
    return np.ascontiguousarray(np.stack([xh, xl], axis=0))


def _prep_w(gate_w):
    import ml_dtypes

    bf = ml_dtypes.bfloat16
    wh = gate_w.astype(bf)
    wl = (gate_w - wh.astype(np.float32)).astype(bf)
    return np.ascontiguousarray(np.concatenate([wh, wl], axis=1))


def _run(x, gate_w, trace=False, trace_cores=None):
    from concourse import bass_utils

    nc = _get_nc()
    x = np.asarray(x, dtype=np.float32)
    gate_w = np.asarray(gate_w, dtype=np.float32)

    whl = _prep_w(gate_w)
    in_maps = [
        {"xhl": _prep_core(x[c * NP:(c + 1) * NP]), "whl": whl}
        for c in range(N_CORES)
    ]
    kw = {}
    if trace:
        kw = {"trace": True,
              "trace_cores": trace_cores if trace_cores is not None else [0]}
    res = bass_utils.run_bass_kernel_spmd(nc, in_maps, list(range(N_CORES)), **kw)
    outs = res.results

    w = np.concatenate([outs[c]["w_out"] for c in range(N_CORES)], axis=0)
    idx = np.concatenate([outs[c]["i_out"] for c in range(N_CORES)],
                         axis=0).astype(np.int32)
    partials = np.stack([outs[c]["p_out"][0] for c in range(N_CORES)], axis=0)
    totals = partials.sum(axis=0, dtype=np.float64)
    counts = totals[:NUM_EXPERTS]
    psum = totals[NUM_EXPERTS:]
    f = counts / float(N_TOKENS)
    P = psum / float(N_TOKENS)
    loss = np.float32(NUM_EXPERTS * np.sum(f * P))
    return (w.astype(np.float32), idx, loss), res


def kernel(x, gate_w):
    (w, idx, loss), _ = _run(x, gate_w)
    return w, idx, loss
